# revision 1
# baseline (speedup 1.0000x reference)
"""GPT block (LN -> causal MHA -> LN -> MLP) on 8 TRN2 NeuronCores.

Sharding: each core owns one (batch, query-chunk-pair). B=4 batches x 2
chunk-pairs = 8 cores. Chunk pairs are zig-zag ({0,3} / {1,2}) over four
512-row chunks of T=2048 so attention work balances. Each core recomputes
K/V for the full sequence locally (no collectives), runs flash-style
attention for its 1024 query rows, then the MLP for the same rows.

All activations live feature-on-partition ("transposed"); the host
pre-transposes x and assembles the output, so no on-chip transposes are
needed. Per-core causality is handled with a block permutation of the
sequence: each core sees [own-chunk-A | own-chunk-B | other | other], so
the program is identical across cores; the diagonal-block masks are
static and full-block allow/deny is driven by per-core exp scale/bias
inputs (exp(0*s - 1e9) = 0 kills forbidden blocks).
"""

import numpy as np
import ml_dtypes

BF = ml_dtypes.bfloat16

E = 1024          # embedding
T = 2048          # sequence
B = 4             # batch
NH = 16           # heads
D = 64            # head dim
HID = 4096        # mlp hidden
KT = E // 128     # k-tiles over embedding (8)
CH = 512          # chunk rows
NEG = -1.0e9
EPS = 1e-5

_CACHE = {}


def _build_program():
    import concourse.bass as bass
    import concourse.tile as tile
    from concourse import bacc, mybir

    f32 = mybir.dt.float32
    bf16 = mybir.dt.bfloat16
    AF = mybir.ActivationFunctionType
    ALU = mybir.AluOpType

    nc = bacc.Bacc()

    xT_d = nc.declare_dram_parameter("xT", [E, T], f32, isOutput=False)
    w_attn_d = nc.declare_dram_parameter("w_attn", [E, 3 * E], bf16, isOutput=False)
    b_qk_d = nc.declare_dram_parameter("b_qk", [2 * E, 1], f32, isOutput=False)
    b_v_d = nc.declare_dram_parameter("b_v", [1, E], f32, isOutput=False)
    w_ap_d = nc.declare_dram_parameter("w_ap", [E, E], bf16, isOutput=False)
    b_ap_d = nc.declare_dram_parameter("b_ap", [E, 1], f32, isOutput=False)
    ln1_g_d = nc.declare_dram_parameter("ln1_g", [E, 1], f32, isOutput=False)
    ln1_b_d = nc.declare_dram_parameter("ln1_b", [E, 1], f32, isOutput=False)
    ln2_g_d = nc.declare_dram_parameter("ln2_g", [E, 1], f32, isOutput=False)
    ln2_b_d = nc.declare_dram_parameter("ln2_b", [E, 1], f32, isOutput=False)
    w_fc_d = nc.declare_dram_parameter("w_fc", [E, HID], bf16, isOutput=False)
    b_fc_d = nc.declare_dram_parameter("b_fc", [HID, 1], f32, isOutput=False)
    w_pr_d = nc.declare_dram_parameter("w_proj", [HID, E], bf16, isOutput=False)
    b_pr_d = nc.declare_dram_parameter("b_proj", [E, 1], f32, isOutput=False)
    dmask_d = nc.declare_dram_parameter("dmask", [4, 128, CH], f32, isOutput=False)
    sA_s_d = nc.declare_dram_parameter("sA_scale", [128, 1], f32, isOutput=False)
    sA_b_d = nc.declare_dram_parameter("sA_bias", [128, 1], f32, isOutput=False)
    sB_s_d = nc.declare_dram_parameter("sB_scale", [128, 3], f32, isOutput=False)
    sB_b_d = nc.declare_dram_parameter("sB_bias", [128, 3], f32, isOutput=False)
    out_d = nc.declare_dram_parameter("outT", [E, 2 * CH], f32, isOutput=True)

    with tile.TileContext(nc) as tc:
        from contextlib import ExitStack

        stack = ExitStack()
        with stack:
            const = stack.enter_context(tc.tile_pool(name="const", bufs=1))

            ones_col_bf = const.tile([128, 1], bf16)
            nc.vector.memset(ones_col_bf[:], 1.0)
            ones_row_f = const.tile([1, 128], f32)
            nc.vector.memset(ones_row_f[:], 1.0)
            ones_row_bf = const.tile([1, 64], bf16)
            nc.vector.memset(ones_row_bf[:], 1.0)
            eps_t = const.tile([1, 1], f32)
            nc.vector.memset(eps_t[:], EPS)

            dmask = const.tile([128, 4, CH], f32)
            nc.sync.dma_start(dmask[:], dmask_d.rearrange("v p n -> p v n"))
            sA_s = const.tile([128, 1], f32)
            nc.sync.dma_start(sA_s[:], sA_s_d[:])
            sA_b = const.tile([128, 1], f32)
            nc.sync.dma_start(sA_b[:], sA_b_d[:])
            sB_s = const.tile([128, 3], f32)
            nc.sync.dma_start(sB_s[:], sB_s_d[:])
            sB_b = const.tile([128, 3], f32)
            nc.sync.dma_start(sB_b[:], sB_b_d[:])

            ln1_g = const.tile([128, KT, 1], f32)
            nc.sync.dma_start(ln1_g[:], ln1_g_d.rearrange("(k p) o -> p k o", p=128))
            ln1_b = const.tile([128, KT, 1], f32)
            nc.sync.dma_start(ln1_b[:], ln1_b_d.rearrange("(k p) o -> p k o", p=128))
            ln2_g = const.tile([128, KT, 1], f32)
            nc.sync.dma_start(ln2_g[:], ln2_g_d.rearrange("(k p) o -> p k o", p=128))
            ln2_b = const.tile([128, KT, 1], f32)
            nc.sync.dma_start(ln2_b[:], ln2_b_d.rearrange("(k p) o -> p k o", p=128))
            b_qk = const.tile([128, 16, 1], f32)
            nc.sync.dma_start(b_qk[:], b_qk_d.rearrange("(k p) o -> p k o", p=128))
            b_v = const.tile([1, E], f32)
            nc.sync.dma_start(b_v[:], b_v_d[:])
            b_ap = const.tile([128, KT, 1], f32)
            nc.sync.dma_start(b_ap[:], b_ap_d.rearrange("(k p) o -> p k o", p=128))
            b_fc = const.tile([128, 32, 1], f32)
            nc.sync.dma_start(b_fc[:], b_fc_d.rearrange("(k p) o -> p k o", p=128))
            b_pr = const.tile([128, KT, 1], f32)
            nc.sync.dma_start(b_pr[:], b_pr_d.rearrange("(k p) o -> p k o", p=128))

            def layernorm(dst, src_fn, ncols, g_t, b_t):
                """dst[:, kt, c] (bf16) = LN over feature dim of src columns.

                src_fn(kt, ch) -> ([128, CH] f32 AP, needs_dma: bool src is DRAM AP)
                """
                with tc.tile_pool(name="lnp", bufs=2, space="PSUM") as lnps, \
                     tc.tile_pool(name="lns", bufs=3) as lnsb:
                    for ch in range(ncols // CH):
                        xts = []
                        mu_ps = lnps.tile([1, CH], f32, tag="stat")
                        ss_ps = lnps.tile([1, CH], f32, tag="stat")
                        for kt in range(KT):
                            src, needs_dma = src_fn(kt, ch)
                            if needs_dma:
                                xt = lnsb.tile([128, CH], f32, tag="xin",
                                               bufs=10)
                                nc.sync.dma_start(xt[:], src)
                            else:
                                xt = src
                            xts.append(xt)
                            xbf = lnsb.tile([128, CH], bf16, tag="xbf")
                            nc.vector.tensor_copy(xbf[:], xt[:])
                            sq = lnsb.tile([128, CH], bf16, tag="sq")
                            nc.scalar.square(sq[:], xbf[:])
                            nc.tensor.matmul(mu_ps[:], ones_col_bf[:], xbf[:],
                                             start=(kt == 0), stop=(kt == KT - 1))
                            nc.tensor.matmul(ss_ps[:], ones_col_bf[:], sq[:],
                                             start=(kt == 0), stop=(kt == KT - 1))
                        mu = lnsb.tile([1, CH], f32, tag="row", bufs=8)
                        nc.scalar.mul(mu[:], mu_ps[:], 1.0 / E)
                        ms = lnsb.tile([1, CH], f32, tag="row", bufs=8)
                        nc.scalar.mul(ms[:], ss_ps[:], 1.0 / E)
                        musq = lnsb.tile([1, CH], f32, tag="row", bufs=8)
                        nc.scalar.square(musq[:], mu[:])
                        var = lnsb.tile([1, CH], f32, tag="row", bufs=8)
                        nc.vector.tensor_sub(var[:], ms[:], musq[:])
                        sd = lnsb.tile([1, CH], f32, tag="row", bufs=8)
                        nc.scalar.activation(sd[:], var[:], AF.Sqrt,
                                             bias=eps_t[:])
                        a_row = lnsb.tile([1, CH], f32, tag="row", bufs=8)
                        nc.vector.reciprocal(a_row[:], sd[:])
                        nmu = lnsb.tile([1, CH], f32, tag="row", bufs=8)
                        nc.scalar.mul(nmu[:], mu[:], -1.0)
                        c_row = lnsb.tile([1, CH], f32, tag="row", bufs=8)
                        nc.vector.tensor_mul(c_row[:], nmu[:], a_row[:])
                        a_bc = lnps.tile([128, CH], f32, tag="bc")
                        nc.tensor.matmul(a_bc[:], ones_row_f[:], a_row[:],
                                         start=True, stop=True)
                        c_bc = lnps.tile([128, CH], f32, tag="bc")
                        nc.tensor.matmul(c_bc[:], ones_row_f[:], c_row[:],
                                         start=True, stop=True)
                        c_sb = lnsb.tile([128, CH], f32, tag="csb")
                        nc.vector.tensor_copy(c_sb[:], c_bc[:])
                        for kt in range(KT):
                            t1 = lnsb.tile([128, CH], f32, tag="t1")
                            nc.vector.tensor_mul(t1[:], xts[kt][:], a_bc[:])
                            t2 = lnsb.tile([128, CH], f32, tag="t2")
                            nc.vector.tensor_add(t2[:], t1[:], c_sb[:])
                            nc.vector.tensor_scalar(
                                dst[:, kt, ch * CH:(ch + 1) * CH], t2[:],
                                g_t[:, kt, 0:1], b_t[:, kt, 0:1],
                                ALU.mult, ALU.add)

            # ---------------- Phase 1+2: LN1 and QKV ----------------
            ln1_ctx = ExitStack()
            ln1 = ln1_ctx.enter_context(tc.tile_pool(name="ln1", bufs=1))
            ln1_t = ln1.tile([128, KT, T], bf16)
            layernorm(ln1_t,
                      lambda kt, ch: (xT_d[kt * 128:(kt + 1) * 128,
                                           ch * CH:(ch + 1) * CH], True),
                      T, ln1_g, ln1_b)

            qkv_ctx = ExitStack()
            qkvp = qkv_ctx.enter_context(tc.tile_pool(name="qkvp", bufs=1, side="right"))
            qT = qkvp.tile([128, KT, 2 * CH], bf16)
            kT = qkvp.tile([128, KT, T], bf16)
            v_aug = qkvp.tile([128, 16, NH * 65], bf16)
            v4 = v_aug.rearrange("p m (h w) -> p m h w", h=NH)

            with tc.tile_pool(name="wqk", bufs=2) as wqk_pool, \
                 tc.tile_pool(name="qkps", bufs=4, space="PSUM") as qkps:
                # Q (groups 0..3 cover cols 0..1023), K (4..11 -> 1024..3071)
                for g in range(8):
                    panel = wqk_pool.tile([128, KT, 256], bf16, tag="w")
                    nc.sync.dma_start(
                        panel[:],
                        w_attn_d.rearrange("(k p) n -> p k n", p=128)[
                            :, :, g * 256:(g + 1) * 256])
                    for mm in range(2):
                        mt = 2 * g + mm
                        is_q = mt < 8
                        n_chunks = 2 if is_q else 4
                        dst = qT if is_q else kT
                        dt_idx = mt if is_q else mt - 8
                        for nq in range(n_chunks):
                            ps = qkps.tile([128, CH], f32, tag="ps")
                            for kt in range(KT):
                                nc.tensor.matmul(
                                    ps[:], panel[:, kt, mm * 128:(mm + 1) * 128],
                                    ln1_t[:, kt, nq * CH:(nq + 1) * CH],
                                    start=(kt == 0), stop=(kt == KT - 1))
                            nc.vector.tensor_scalar(
                                dst[:, dt_idx, nq * CH:(nq + 1) * CH], ps[:],
                                b_qk[:, mt, 0:1], None, ALU.add)
                # V in natural layout, heads interleaved with a ones column
                for g in range(2):
                    panel = wqk_pool.tile([128, KT, CH], bf16, tag="wv")
                    nc.sync.dma_start(
                        panel[:],
                        w_attn_d.rearrange("(k p) n -> p k n", p=128)[
                            :, :, 2 * E + g * CH:2 * E + (g + 1) * CH])
                    bv_bc = qkps.tile([128, CH], f32, tag="bv", bufs=2)
                    nc.tensor.matmul(bv_bc[:], ones_row_f[:],
                                     b_v[:, g * CH:(g + 1) * CH],
                                     start=True, stop=True)
                    bv_sb = wqk_pool.tile([128, CH], f32, tag="bvs")
                    nc.vector.tensor_copy(bv_sb[:], bv_bc[:])
                    bv_sb3 = bv_sb.rearrange("p (h w) -> p h w", h=8)
                    for mv in range(16):
                        ps = qkps.tile([128, CH], f32, tag="ps")
                        for kt in range(KT):
                            nc.tensor.matmul(
                                ps[:], ln1_t[:, kt, mv * 128:(mv + 1) * 128],
                                panel[:, kt, :],
                                start=(kt == 0), stop=(kt == KT - 1))
                        ps3 = ps.rearrange("p (h w) -> p h w", h=8)
                        nc.vector.tensor_add(
                            v4[:, mv, g * 8:(g + 1) * 8, 0:64], ps3[:], bv_sb3[:])
                for mv in range(16):
                    nc.vector.memset(v4[:, mv, :, 64:65], 1.0)
            ln1_ctx.close()

            # ---------------- Phase 3: attention ----------------
            attn_ctx = ExitStack()
            attnp = attn_ctx.enter_context(tc.tile_pool(name="attnp", bufs=1))
            attnT = attnp.tile([128, KT, 2 * CH], bf16)

            # (kv_tile, mask) sequences; mask: ("diag", r) | ("drv", which, idx)
            seq_a = [(t, ("diag", t)) for t in range(4)] + \
                    [(8 + t, ("drv", "A", 0)) for t in range(4)]
            seq_b = [(4 + t, ("diag", t)) for t in range(4)] + \
                    [(t, ("drv", "B", 0)) for t in range(4)] + \
                    [(8 + t, ("drv", "B", 1)) for t in range(4)] + \
                    [(12 + t, ("drv", "B", 2)) for t in range(4)]

            with tc.tile_pool(name="atps", bufs=1, space="PSUM") as atps, \
                 tc.tile_pool(name="atsb", bufs=4) as atsb:
                for h in range(NH):
                    ktf = h // 2
                    ro = (h % 2) * 64
                    for slot, seq in ((0, seq_a), (1, seq_b)):
                        qc = slice(slot * CH, (slot + 1) * CH)
                        out_ps = atps.tile([65, CH], f32, tag="av", bufs=2)
                        last = len(seq) - 1
                        for i, (kvt, mk) in enumerate(seq):
                            s_ps = atps.tile([128, CH], f32, tag="s", bufs=4)
                            nc.tensor.matmul(
                                s_ps[:],
                                kT[ro:ro + 64, ktf, kvt * 128:(kvt + 1) * 128],
                                qT[ro:ro + 64, ktf, qc],
                                start=True, stop=True)
                            es = atsb.tile([128, CH], bf16, tag="es")
                            if mk[0] == "diag":
                                tmp = atsb.tile([128, CH], f32, tag="tmp")
                                nc.vector.tensor_add(
                                    tmp[:], s_ps[:], dmask[:, mk[1], :])
                                nc.scalar.activation(es[:], tmp[:], AF.Exp)
                            else:
                                sc = sA_s if mk[1] == "A" else sB_s
                                bi = sA_b if mk[1] == "A" else sB_b
                                idx = mk[2]
                                nc.scalar.activation(
                                    es[:], s_ps[:], AF.Exp,
                                    bias=bi[:, idx:idx + 1],
                                    scale=sc[:, idx:idx + 1])
                            nc.tensor.matmul(
                                out_ps[:], v_aug[:, kvt, h * 65:(h + 1) * 65],
                                es[:], start=(i == 0), stop=(i == last))
                        recip = atsb.tile([1, CH], bf16, tag="rc")
                        with nc.allow_low_precision(
                                reason="softmax denom reciprocal in bf16"):
                            nc.vector.reciprocal(recip[:], out_ps[64:65, :])
                        bc_ps = atps.tile([64, CH], f32, tag="bc", bufs=2)
                        nc.tensor.matmul(bc_ps[:], ones_row_bf[:], recip[:],
                                         start=True, stop=True)
                        bc_sb = atsb.tile([64, CH], f32, tag="bcs")
                        nc.vector.tensor_copy(bc_sb[:], bc_ps[:])
                        nc.vector.tensor_mul(
                            attnT[ro:ro + 64, ktf, qc], out_ps[0:64, :], bc_sb[:])

            qkv_ctx.close()

            # ---------------- Phase 4: attn proj + residual ----------------
            x2p = stack.enter_context(tc.tile_pool(name="x2p", bufs=1, side="right"))
            x2T = x2p.tile([128, KT, 2 * CH], f32)

            with tc.tile_pool(name="app", bufs=1) as app, \
                 tc.tile_pool(name="apsb", bufs=3) as apsb, \
                 tc.tile_pool(name="apps", bufs=3, space="PSUM") as apps:
                w_ap = app.tile([128, KT, E], bf16)
                nc.sync.dma_start(w_ap[:], w_ap_d.rearrange("(k p) n -> p k n", p=128))
                for m in range(KT):
                    for nq in range(2):
                        ps = apps.tile([128, CH], f32, tag="ps")
                        for kt in range(KT):
                            nc.tensor.matmul(
                                ps[:], w_ap[:, kt, m * 128:(m + 1) * 128],
                                attnT[:, kt, nq * CH:(nq + 1) * CH],
                                start=(kt == 0), stop=(kt == KT - 1))
                        xq = apsb.tile([128, CH], f32, tag="xq")
                        nc.sync.dma_start(
                            xq[:], xT_d[m * 128:(m + 1) * 128,
                                        nq * CH:(nq + 1) * CH])
                        nc.vector.scalar_tensor_tensor(
                            x2T[:, m, nq * CH:(nq + 1) * CH], ps[:],
                            b_ap[:, m, 0:1], xq[:], ALU.add, ALU.add)

            attn_ctx.close()

            # ---------------- Phase 5: LN2 ----------------
            h2_ctx = ExitStack()
            h2p = h2_ctx.enter_context(tc.tile_pool(name="h2p", bufs=1))
            h2T = h2p.tile([128, KT, 2 * CH], bf16)
            layernorm(h2T,
                      lambda kt, ch: (x2T[:, kt, ch * CH:(ch + 1) * CH], False),
                      2 * CH, ln2_g, ln2_b)

            # ---------------- Phase 6: FC + GELU ----------------
            gp = stack.enter_context(tc.tile_pool(name="gp", bufs=1, side="right"))
            gT = gp.tile([128, 32, 2 * CH], bf16)

            with tc.tile_pool(name="wfcp", bufs=2) as wfcp, \
                 tc.tile_pool(name="fcps", bufs=4, space="PSUM") as fcps:
                for mg in range(8):
                    panel = wfcp.tile([128, KT, CH], bf16, tag="w")
                    nc.sync.dma_start(
                        panel[:],
                        w_fc_d.rearrange("(k p) n -> p k n", p=128)[
                            :, :, mg * CH:(mg + 1) * CH])
                    for mm in range(4):
                        mt = mg * 4 + mm
                        for nq in range(2):
                            ps = fcps.tile([128, CH], f32, tag="ps")
                            for kt in range(KT):
                                nc.tensor.matmul(
                                    ps[:], panel[:, kt, mm * 128:(mm + 1) * 128],
                                    h2T[:, kt, nq * CH:(nq + 1) * CH],
                                    start=(kt == 0), stop=(kt == KT - 1))
                            nc.scalar.activation(
                                gT[:, mt, nq * CH:(nq + 1) * CH], ps[:],
                                AF.Gelu, bias=b_fc[:, mt, 0:1])

            h2_ctx.close()

            # ---------------- Phase 7: proj + residual + out ----------------
            with tc.tile_pool(name="wprp", bufs=3) as wprp, \
                 tc.tile_pool(name="prsb", bufs=3) as prsb, \
                 tc.tile_pool(name="prps", bufs=8, space="PSUM") as prps:
                for nq in range(2):
                    pss = [prps.tile([128, CH], f32, tag="ps", name=f"prps{m}")
                           for m in range(KT)]
                    for kt in range(32):
                        panel = wprp.tile([128, E], bf16, tag="w")
                        nc.sync.dma_start(
                            panel[:], w_pr_d[kt * 128:(kt + 1) * 128, :])
                        for m in range(KT):
                            nc.tensor.matmul(
                                pss[m][:], panel[:, m * 128:(m + 1) * 128],
                                gT[:, kt, nq * CH:(nq + 1) * CH],
                                start=(kt == 0), stop=(kt == 31),
                                skip_group_check=True)
                    for m in range(KT):
                        ot = prsb.tile([128, CH], f32, tag="ot")
                        nc.vector.scalar_tensor_tensor(
                            ot[:], pss[m][:], b_pr[:, m, 0:1],
                            x2T[:, m, nq * CH:(nq + 1) * CH],
                            ALU.add, ALU.add)
                        nc.sync.dma_start(
                            out_d[m * 128:(m + 1) * 128, nq * CH:(nq + 1) * CH],
                            ot[:])

    nc.compile()
    return nc


def _host_prep(inputs):
    """Build the 8 per-core input maps."""
    x = np.asarray(inputs["x"], np.float32)
    w_attn = np.asarray(inputs["w_attn"], np.float32).copy()
    w_attn[:, :E] *= 0.125  # fold 1/sqrt(head_dim) into Q
    b_attn = np.asarray(inputs["b_attn"], np.float32).copy()
    b_attn[:E] *= 0.125
    w_attn_bf = np.ascontiguousarray(w_attn.astype(BF))
    b_qk = np.ascontiguousarray(b_attn[:2 * E].reshape(2 * E, 1))
    b_v = np.ascontiguousarray(b_attn[2 * E:].reshape(1, E))
    w_ap_bf = np.ascontiguousarray(np.asarray(inputs["w_attnproj"], np.float32).astype(BF))
    w_fc_bf = np.ascontiguousarray(np.asarray(inputs["w_fc"], np.float32).astype(BF))
    w_pr_bf = np.ascontiguousarray(np.asarray(inputs["w_proj"], np.float32).astype(BF))
    col = lambda v: np.ascontiguousarray(np.asarray(v, np.float32).reshape(-1, 1))
    b_ap = col(inputs["b_attnproj"])
    b_fc = col(inputs["b_fc"])
    b_pr = col(inputs["b_proj"])
    ln1_g = col(inputs["ln1_g"]); ln1_b = col(inputs["ln1_b"])
    ln2_g = col(inputs["ln2_g"]); ln2_b = col(inputs["ln2_b"])

    # static diagonal masks: dmask[r][p, j] = 0 if j >= r*128+p else NEG
    j = np.arange(CH)[None, :]
    p = np.arange(128)[:, None]
    dmask = np.stack([np.where(j >= r * 128 + p, 0.0, NEG).astype(np.float32)
                      for r in range(4)])
    dmask = np.ascontiguousarray(dmask)

    ON = (1.0, 0.0)
    OFF = (0.0, NEG)
    in_maps = []
    perms = []
    for core in range(8):
        b = core // 2
        z = core % 2
        blocks = [0, 3, 1, 2] if z == 0 else [1, 2, 0, 3]
        perms.append(blocks)
        cols = np.concatenate([np.arange(c * CH, (c + 1) * CH) for c in blocks])
        xT = np.ascontiguousarray(x[b].T[:, cols])
        # slot A: driven block = O1 (perm pos 2); allowed iff block(O1) < block(A)
        sa = ON if blocks[2] < blocks[0] else OFF
        # slot B: driven = A, O1, O2 (perm pos 0, 2, 3) vs chunk B
        sbs = [ON if blocks[i] < blocks[1] else OFF for i in (0, 2, 3)]
        f = np.float32
        in_maps.append({
            "xT": xT,
            "w_attn": w_attn_bf, "b_qk": b_qk, "b_v": b_v,
            "w_ap": w_ap_bf, "b_ap": b_ap,
            "ln1_g": ln1_g, "ln1_b": ln1_b, "ln2_g": ln2_g, "ln2_b": ln2_b,
            "w_fc": w_fc_bf, "b_fc": b_fc, "w_proj": w_pr_bf, "b_proj": b_pr,
            "dmask": dmask,
            "sA_scale": np.full((128, 1), sa[0], f),
            "sA_bias": np.full((128, 1), sa[1], f),
            "sB_scale": np.ascontiguousarray(
                np.tile(np.array([[s for s, _ in sbs]], f), (128, 1))),
            "sB_bias": np.ascontiguousarray(
                np.tile(np.array([[bb for _, bb in sbs]], f), (128, 1))),
        })
    return in_maps, perms


def _run(inputs, trace=False):
    from concourse.bass_utils import run_bass_kernel_spmd

    if "nc" not in _CACHE:
        _CACHE["nc"] = _build_program()
    nc = _CACHE["nc"]
    in_maps, perms = _host_prep(inputs)
    res = run_bass_kernel_spmd(nc, in_maps, list(range(8)), trace=trace)
    x = np.asarray(inputs["x"], np.float32)
    out = np.empty_like(x)
    for core in range(8):
        b = core // 2
        blocks = perms[core]
        oT = res.results[core]["outT"]
        cA, cB = blocks[0], blocks[1]
        out[b, cA * CH:(cA + 1) * CH, :] = oT[:, 0:CH].T
        out[b, cB * CH:(cB + 1) * CH, :] = oT[:, CH:2 * CH].T
    return out, res


def kernel(**inputs) -> np.ndarray:
    out, _ = _run(inputs, trace=False)
    return out



# revision 17
# speedup vs baseline: 1.3427x; 1.3427x over previous
"""GPT block (LN -> causal MHA -> LN -> MLP) on 8 TRN2 NeuronCores.

Sharding: each core owns one (batch, query-chunk-pair). B=4 batches x 2
chunk-pairs = 8 cores. Chunk pairs are zig-zag ({0,3} / {1,2}) over four
512-row chunks of T=2048 so attention work balances. Each core recomputes
K/V for the full sequence locally (no collectives), runs flash-style
attention for its 1024 query rows, then the MLP for the same rows.

All activations live feature-on-partition ("transposed"); the host
pre-transposes x and assembles the output, so no on-chip transposes are
needed. Per-core causality is handled with a block permutation of the
sequence: each core sees [own-chunk-A | own-chunk-B | other | other], so
the program is identical across cores; the diagonal-block masks are
static and full-block allow/deny is driven by per-core exp scale/bias
inputs (exp(0*s - 1e9) = 0 kills forbidden blocks).
"""

import numpy as np
import ml_dtypes

BF = ml_dtypes.bfloat16

E = 1024          # embedding
T = 2048          # sequence
B = 4             # batch
NH = 16           # heads
D = 64            # head dim
HID = 4096        # mlp hidden
KT = E // 128     # k-tiles over embedding (8)
CH = 512          # chunk rows
NEG = -1.0e9
EPS = 1e-5

_CACHE = {}


def _build_program():
    import concourse.bass as bass
    import concourse.tile as tile
    from concourse import bacc, mybir

    f32 = mybir.dt.float32
    bf16 = mybir.dt.bfloat16
    f8 = mybir.dt.float8e4
    AF = mybir.ActivationFunctionType
    ALU = mybir.AluOpType
    DR = mybir.MatmulPerfMode.DoubleRow

    nc = bacc.Bacc()

    xT_d = nc.declare_dram_parameter("xT", [E, T], f32, isOutput=False)
    w_attn_d = nc.declare_dram_parameter("w_attn", [E, 3 * E], f8, isOutput=False)
    b_qk_d = nc.declare_dram_parameter("b_qk", [2 * E, 1], f32, isOutput=False)
    b_v_d = nc.declare_dram_parameter("b_v", [1, E], f32, isOutput=False)
    w_ap_d = nc.declare_dram_parameter("w_ap", [E, E], f8, isOutput=False)
    b_ap_d = nc.declare_dram_parameter("b_ap", [1, E], bf16, isOutput=False)
    ln1_g_d = nc.declare_dram_parameter("ln1_g", [E, 1], f32, isOutput=False)
    ln1_b_d = nc.declare_dram_parameter("ln1_b", [E, 1], f32, isOutput=False)
    ln2_g_d = nc.declare_dram_parameter("ln2_g", [E, 1], f32, isOutput=False)
    ln2_b_d = nc.declare_dram_parameter("ln2_b", [E, 1], f32, isOutput=False)
    w_fc_d = nc.declare_dram_parameter("w_fc", [E, HID], f8, isOutput=False)
    b_fc_d = nc.declare_dram_parameter("b_fc", [HID, 1], f32, isOutput=False)
    w_pr_d = nc.declare_dram_parameter("w_proj", [HID, E], f8, isOutput=False)
    b_pr_d = nc.declare_dram_parameter("b_proj", [1, E], bf16, isOutput=False)
    dmask_d = nc.declare_dram_parameter("dmask", [4, 128, CH], bf16, isOutput=False)
    sA_s_d = nc.declare_dram_parameter("sA_scale", [128, 1], f32, isOutput=False)
    sA_b_d = nc.declare_dram_parameter("sA_bias", [128, 1], f32, isOutput=False)
    sB_s_d = nc.declare_dram_parameter("sB_scale", [128, 3], f32, isOutput=False)
    sB_b_d = nc.declare_dram_parameter("sB_bias", [128, 3], f32, isOutput=False)
    out_d = nc.declare_dram_parameter("outT", [E, 2 * CH], f32, isOutput=True)

    with tile.TileContext(nc) as tc:
        from contextlib import ExitStack

        stack = ExitStack()
        with stack:
            const = stack.enter_context(tc.tile_pool(name="const", bufs=1))

            ones_col_bf = const.tile([128, 1], bf16)
            nc.vector.memset(ones_col_bf[:], 1.0)
            ones_row_f = const.tile([1, 128], f32)
            nc.vector.memset(ones_row_f[:], 1.0)
            ones_row_bf = const.tile([1, 64], bf16)
            nc.vector.memset(ones_row_bf[:], 1.0)
            ones_ch_bf = const.tile([1, CH], bf16)
            nc.vector.memset(ones_ch_bf[:], 1.0)
            eps_t = const.tile([1, 1], f32)
            nc.vector.memset(eps_t[:], EPS)

            dmask = const.tile([128, 4, CH], bf16)
            nc.sync.dma_start(dmask[:], dmask_d.rearrange("v p n -> p v n"))
            sA_s = const.tile([128, 1], f32)
            nc.sync.dma_start(sA_s[:], sA_s_d[:])
            sA_b = const.tile([128, 1], f32)
            nc.sync.dma_start(sA_b[:], sA_b_d[:])
            sB_s = const.tile([128, 3], f32)
            nc.sync.dma_start(sB_s[:], sB_s_d[:])
            sB_b = const.tile([128, 3], f32)
            nc.sync.dma_start(sB_b[:], sB_b_d[:])

            ln1_g = const.tile([128, KT, 1], f32)
            nc.sync.dma_start(ln1_g[:], ln1_g_d.rearrange("(k p) o -> p k o", p=128))
            ln1_b = const.tile([128, KT, 1], f32)
            nc.sync.dma_start(ln1_b[:], ln1_b_d.rearrange("(k p) o -> p k o", p=128))
            ln2_g = const.tile([128, KT, 1], f32)
            nc.sync.dma_start(ln2_g[:], ln2_g_d.rearrange("(k p) o -> p k o", p=128))
            ln2_b = const.tile([128, KT, 1], f32)
            nc.sync.dma_start(ln2_b[:], ln2_b_d.rearrange("(k p) o -> p k o", p=128))
            b_qk = const.tile([128, 16, 1], f32)
            nc.sync.dma_start(b_qk[:], b_qk_d.rearrange("(k p) o -> p k o", p=128))
            b_v = const.tile([1, E], f32)
            nc.sync.dma_start(b_v[:], b_v_d[:])
            b_ap = const.tile([1, E], bf16)
            nc.sync.dma_start(b_ap[:], b_ap_d[:])
            b_fc = const.tile([128, 32, 1], f32)
            nc.sync.dma_start(b_fc[:], b_fc_d.rearrange("(k p) o -> p k o", p=128))
            b_pr = const.tile([1, E], bf16)
            nc.sync.dma_start(b_pr[:], b_pr_d[:])

            def layernorm(dst, src_fn, ncols, g_t, b_t):
                """dst[:, kt, c] (bf16) = LN over feature dim of src columns.

                src_fn(kt, ch) -> ([128, CH] f32 AP, needs_dma: bool src is DRAM AP)
                """
                with tc.tile_pool(name="lnp", bufs=2, space="PSUM") as lnps, \
                     tc.tile_pool(name="lns", bufs=3) as lnsb:
                    for ch in range(ncols // CH):
                        xts = []
                        mu_ps = lnps.tile([1, CH], f32, tag="stat")
                        ss_ps = lnps.tile([1, CH], f32, tag="stat")
                        for kt in range(KT):
                            src, needs_dma = src_fn(kt, ch)
                            if needs_dma:
                                xt = lnsb.tile([128, CH], f32, tag="xin",
                                               bufs=10)
                                nc.sync.dma_start(xt[:], src)
                            else:
                                xt = src
                            xts.append(xt)
                            xbf = lnsb.tile([128, CH], bf16, tag="xbf")
                            nc.vector.tensor_copy(xbf[:], xt[:])
                            sq = lnsb.tile([128, CH], bf16, tag="sq")
                            nc.scalar.square(sq[:], xbf[:])
                            nc.tensor.matmul(mu_ps[:], ones_col_bf[:], xbf[:],
                                             start=(kt == 0), stop=(kt == KT - 1))
                            nc.tensor.matmul(ss_ps[:], ones_col_bf[:], sq[:],
                                             start=(kt == 0), stop=(kt == KT - 1))
                        mu = lnsb.tile([1, CH], f32, tag="row", bufs=8)
                        nc.scalar.mul(mu[:], mu_ps[:], 1.0 / E)
                        ms = lnsb.tile([1, CH], f32, tag="row", bufs=8)
                        nc.scalar.mul(ms[:], ss_ps[:], 1.0 / E)
                        musq = lnsb.tile([1, CH], f32, tag="row", bufs=8)
                        nc.scalar.square(musq[:], mu[:])
                        var = lnsb.tile([1, CH], f32, tag="row", bufs=8)
                        nc.vector.tensor_sub(var[:], ms[:], musq[:])
                        sd = lnsb.tile([1, CH], f32, tag="row", bufs=8)
                        nc.scalar.activation(sd[:], var[:], AF.Sqrt,
                                             bias=eps_t[:])
                        a_row = lnsb.tile([1, CH], f32, tag="row", bufs=8)
                        nc.vector.reciprocal_approx_fast(out=a_row[:], in_=sd[:])
                        nmu = lnsb.tile([1, CH], f32, tag="row", bufs=8)
                        nc.scalar.mul(nmu[:], mu[:], -1.0)
                        c_row = lnsb.tile([1, CH], f32, tag="row", bufs=8)
                        nc.vector.tensor_mul(c_row[:], nmu[:], a_row[:])
                        a_bc = lnps.tile([128, CH], f32, tag="bc")
                        nc.tensor.matmul(a_bc[:], ones_row_f[:], a_row[:],
                                         start=True, stop=True)
                        c_bc = lnps.tile([128, CH], f32, tag="bc")
                        nc.tensor.matmul(c_bc[:], ones_row_f[:], c_row[:],
                                         start=True, stop=True)
                        c_sb = lnsb.tile([128, CH], f32, tag="csb")
                        nc.vector.tensor_copy(c_sb[:], c_bc[:])
                        for kt in range(KT):
                            t1 = lnsb.tile([128, CH], f32, tag="t1")
                            nc.vector.tensor_mul(t1[:], xts[kt][:], a_bc[:])
                            t2 = lnsb.tile([128, CH], f32, tag="t2")
                            nc.vector.tensor_add(t2[:], t1[:], c_sb[:])
                            nc.vector.tensor_scalar(
                                dst[:, kt, ch * CH:(ch + 1) * CH], t2[:],
                                g_t[:, kt, 0:1], b_t[:, kt, 0:1],
                                ALU.mult, ALU.add)

            # ---------------- Phase 1+2: LN1 and QKV ----------------
            ln1_ctx = ExitStack()
            ln1 = ln1_ctx.enter_context(tc.tile_pool(name="ln1", bufs=1))
            ln1_t = ln1.tile([128, KT, T], f8)
            layernorm(ln1_t,
                      lambda kt, ch: (xT_d[kt * 128:(kt + 1) * 128,
                                           ch * CH:(ch + 1) * CH], True),
                      T, ln1_g, ln1_b)

            qkv_ctx = ExitStack()
            qkvp = qkv_ctx.enter_context(tc.tile_pool(name="qkvp", bufs=1, side="right"))
            qT = qkvp.tile([128, KT, 2 * CH], bf16)
            kT = qkvp.tile([128, KT, T], bf16)
            v_aug = qkvp.tile([128, 16, NH * 65], bf16)
            v4 = v_aug.rearrange("p m (h w) -> p m h w", h=NH)

            with tc.tile_pool(name="wqk", bufs=2) as wqk_pool, \
                 tc.tile_pool(name="qkps", bufs=4, space="PSUM") as qkps:
                # Q (groups 0..3 cover cols 0..1023), K (4..11 -> 1024..3071)
                # fp8 DoubleRow: contraction pairs of k-tiles, PSUM = 1024x
                # true scale (acts x16, weights x64); descale on evacuation.
                for g in range(8):
                    panel = wqk_pool.tile([128, KT, 256], f8, tag="w")
                    nc.sync.dma_start(
                        panel[:],
                        w_attn_d.rearrange("(k p) n -> p k n", p=128)[
                            :, :, g * 256:(g + 1) * 256])
                    for mm in range(2):
                        mt = 2 * g + mm
                        is_q = mt < 8
                        n_chunks = 2 if is_q else 4
                        dst = qT if is_q else kT
                        dt_idx = mt if is_q else mt - 8
                        for nq in range(n_chunks):
                            ps = qkps.tile([128, CH], f32, tag="ps")
                            for kt in range(0, KT, 2):
                                nc.tensor.matmul(
                                    ps[:],
                                    panel[:, kt:kt + 2, mm * 128:(mm + 1) * 128],
                                    ln1_t[:, kt:kt + 2, nq * CH:(nq + 1) * CH],
                                    start=(kt == 0), stop=(kt == KT - 2),
                                    perf_mode=DR)
                            nc.scalar.activation(
                                dst[:, dt_idx, nq * CH:(nq + 1) * CH], ps[:],
                                AF.Identity, bias=b_qk[:, mt, 0:1],
                                scale=2.0 ** -10)
                # V in natural layout, heads interleaved with a ones column
                for g in range(2):
                    panel = wqk_pool.tile([128, KT, CH], f8, tag="wv")
                    nc.sync.dma_start(
                        panel[:],
                        w_attn_d.rearrange("(k p) n -> p k n", p=128)[
                            :, :, 2 * E + g * CH:2 * E + (g + 1) * CH])
                    bv_bc = qkps.tile([128, CH], f32, tag="bv", bufs=2)
                    nc.tensor.matmul(bv_bc[:], ones_row_f[:],
                                     b_v[:, g * CH:(g + 1) * CH],
                                     start=True, stop=True)
                    bv_sb = wqk_pool.tile([128, CH], f32, tag="bvs")
                    nc.vector.tensor_copy(bv_sb[:], bv_bc[:])
                    bv_sb3 = bv_sb.rearrange("p (h w) -> p h w", h=8)
                    for mv in range(16):
                        ps = qkps.tile([128, CH], f32, tag="ps")
                        for kt in range(0, KT, 2):
                            nc.tensor.matmul(
                                ps[:],
                                ln1_t[:, kt:kt + 2, mv * 128:(mv + 1) * 128],
                                panel[:, kt:kt + 2, :],
                                start=(kt == 0), stop=(kt == KT - 2),
                                perf_mode=DR)
                        ps3 = ps.rearrange("p (h w) -> p h w", h=8)
                        nc.vector.scalar_tensor_tensor(
                            v4[:, mv, g * 8:(g + 1) * 8, 0:64], ps3[:],
                            2.0 ** -10, bv_sb3[:], ALU.mult, ALU.add)
                for mv in range(16):
                    nc.vector.memset(v4[:, mv, :, 64:65], 1.0)
            ln1_ctx.close()

            # ---------------- Phase 3: attention ----------------
            # Head-PAIR processing: heads (2j, 2j+1) live on partition rows
            # 0:64 / 64:128 of feature group j, so their score matmuls use
            # disjoint PE row-groups (tile_position auto-derived) and run
            # concurrently. kv tiles are processed in groups of 2 of the same
            # mask kind so one exp ACTIVATE covers [128, 2*CH] (2 PSUM banks).
            # Diag masks are 0/1 multiplies AFTER exp (cheaper: bf16 2x DVE).
            attn_ctx = ExitStack()
            attnp = attn_ctx.enter_context(tc.tile_pool(name="attnp", bufs=1))
            attnT = attnp.tile([128, KT, 2 * CH], f8)

            # groups: (kv_t0, kv_t1, kind); kind: ("diag", pair) | ("gate", which, idx)
            groups_a = [(0, 1, ("diag", 0)), (2, 3, ("diag", 1)),
                        (8, 9, ("gate", "A", 0)), (10, 11, ("gate", "A", 0))]
            groups_b = [(4, 5, ("diag", 0)), (6, 7, ("diag", 1)),
                        (0, 1, ("gate", "B", 0)), (2, 3, ("gate", "B", 0)),
                        (8, 9, ("gate", "B", 1)), (10, 11, ("gate", "B", 1)),
                        (12, 13, ("gate", "B", 2)), (14, 15, ("gate", "B", 2))]

            with tc.tile_pool(name="atps", bufs=1, space="PSUM") as atps, \
                 tc.tile_pool(name="atsb", bufs=1) as atsb:
                for slot, groups in ((0, groups_a), (1, groups_b)):
                    qc = slice(slot * CH, (slot + 1) * CH)
                    last = len(groups) - 1
                    for j in range(8):
                        avs = [atps.tile([65, CH], f32, tag="av", bufs=3,
                                         name=f"av{hh}")
                               for hh in range(2)]
                        for gi, (t0, t1, kind) in enumerate(groups):
                            ss = [atps.tile([128, 2, CH], f32, tag="s",
                                            bufs=2, name=f"s{hh}")
                                  for hh in range(2)]
                            for ti, t in enumerate((t0, t1)):
                                for hh in range(2):
                                    ro = hh * 64
                                    nc.tensor.matmul(
                                        ss[hh][:, ti, :],
                                        kT[ro:ro + 64, j, t * 128:(t + 1) * 128],
                                        qT[ro:ro + 64, j, qc],
                                        start=True, stop=True)
                            for hh in range(2):
                                es = atsb.tile([128, 2, CH], bf16, tag="es",
                                               bufs=4)
                                if kind[0] == "diag":
                                    er = atsb.tile([128, 2, CH], bf16,
                                                   tag="er", bufs=2)
                                    nc.scalar.activation(er[:], ss[hh][:],
                                                         AF.Exp)
                                    pr = kind[1]
                                    nc.vector.tensor_mul(
                                        es[:], er[:],
                                        dmask[:, 2 * pr:2 * pr + 2, :])
                                else:
                                    sc = sA_s if kind[1] == "A" else sB_s
                                    bi = sA_b if kind[1] == "A" else sB_b
                                    idx = kind[2]
                                    nc.scalar.activation(
                                        es[:], ss[hh][:], AF.Exp,
                                        bias=bi[:, idx:idx + 1],
                                        scale=sc[:, idx:idx + 1])
                                h = 2 * j + hh
                                for ti, t in enumerate((t0, t1)):
                                    nc.tensor.matmul(
                                        avs[hh][:],
                                        v_aug[:, t, h * 65:(h + 1) * 65],
                                        es[:, ti, :],
                                        start=(gi == 0 and ti == 0),
                                        stop=(gi == last and ti == 1))
                        # normalization: fast reciprocal of denominator row
                        # (read straight from PSUM), PE broadcast, scale
                        for hh in range(2):
                            ro = hh * 64
                            den = atsb.tile([1, CH], f32, tag="den", bufs=2)
                            nc.vector.tensor_copy(den[:], avs[hh][64:65, :])
                            drc = atsb.tile([1, CH], f32, tag="drc", bufs=2)
                            nc.vector.reciprocal_approx_fast(
                                out=drc[:], in_=den[:])
                            drb = atsb.tile([1, CH], bf16, tag="drb", bufs=2)
                            nc.vector.tensor_copy(drb[:], drc[:])
                            bc_ps = atps.tile([64, CH], f32, tag="bc", bufs=1)
                            nc.tensor.matmul(bc_ps[:], ones_row_bf[:], drb[:],
                                             start=True, stop=True)
                            bc_sb = atsb.tile([64, CH], bf16, tag="bcs",
                                              bufs=2)
                            nc.vector.tensor_copy(bc_sb[:], bc_ps[:])
                            nc.vector.tensor_mul(
                                attnT[ro:ro + 64, j, qc],
                                avs[hh][0:64, :], bc_sb[:])

            qkv_ctx.close()

            # ---------------- Phase 4: attn proj + residual ----------------
            x2p = stack.enter_context(tc.tile_pool(name="x2p", bufs=1, side="right"))
            x2T = x2p.tile([128, KT, 2 * CH], f32)

            with tc.tile_pool(name="app", bufs=1) as app, \
                 tc.tile_pool(name="apsb", bufs=3) as apsb, \
                 tc.tile_pool(name="apps", bufs=3, space="PSUM") as apps:
                w_ap = app.tile([128, KT, E], f8)
                nc.sync.dma_start(w_ap[:], w_ap_d.rearrange("(k p) n -> p k n", p=128))
                for m in range(KT):
                    for nq in range(2):
                        ps = apps.tile([128, CH], f32, tag="ps")
                        for kt in range(0, KT, 2):
                            nc.tensor.matmul(
                                ps[:], w_ap[:, kt:kt + 2, m * 128:(m + 1) * 128],
                                attnT[:, kt:kt + 2, nq * CH:(nq + 1) * CH],
                                start=(kt == 0), stop=False, perf_mode=DR)
                        # bias via rank-1 matmul (bias row is pre-scaled x64)
                        nc.tensor.matmul(
                            ps[:], b_ap[0:1, m * 128:(m + 1) * 128],
                            ones_ch_bf[:], start=False, stop=True,
                            skip_group_check=True)
                        xq = apsb.tile([128, CH], f32, tag="xq")
                        nc.sync.dma_start(
                            xq[:], xT_d[m * 128:(m + 1) * 128,
                                        nq * CH:(nq + 1) * CH])
                        nc.vector.scalar_tensor_tensor(
                            x2T[:, m, nq * CH:(nq + 1) * CH], ps[:],
                            2.0 ** -6, xq[:], ALU.mult, ALU.add)

            attn_ctx.close()

            # ---------------- Phase 5: LN2 ----------------
            h2_ctx = ExitStack()
            h2p = h2_ctx.enter_context(tc.tile_pool(name="h2p", bufs=1))
            h2T = h2p.tile([128, KT, 2 * CH], bf16)
            layernorm(h2T,
                      lambda kt, ch: (x2T[:, kt, ch * CH:(ch + 1) * CH], False),
                      2 * CH, ln2_g, ln2_b)

            # ---------------- Phase 6: FC + GELU ----------------
            gp = stack.enter_context(tc.tile_pool(name="gp", bufs=1, side="right"))
            gT = gp.tile([128, 32, 2 * CH], bf16)

            with tc.tile_pool(name="wfcp", bufs=2) as wfcp, \
                 tc.tile_pool(name="fcps", bufs=4, space="PSUM") as fcps:
                for mg in range(8):
                    panel = wfcp.tile([128, KT, CH], bf16, tag="w")
                    nc.sync.dma_start(
                        panel[:],
                        w_fc_d.rearrange("(k p) n -> p k n", p=128)[
                            :, :, mg * CH:(mg + 1) * CH])
                    for mm in range(4):
                        mt = mg * 4 + mm
                        for nq in range(2):
                            ps = fcps.tile([128, CH], f32, tag="ps")
                            for kt in range(KT):
                                nc.tensor.matmul(
                                    ps[:], panel[:, kt, mm * 128:(mm + 1) * 128],
                                    h2T[:, kt, nq * CH:(nq + 1) * CH],
                                    start=(kt == 0), stop=(kt == KT - 1))
                            nc.scalar.activation(
                                gT[:, mt, nq * CH:(nq + 1) * CH], ps[:],
                                AF.Gelu, bias=b_fc[:, mt, 0:1])

            h2_ctx.close()

            # ---------------- Phase 7: proj + residual + out ----------------
            with tc.tile_pool(name="wprp", bufs=3) as wprp, \
                 tc.tile_pool(name="prsb", bufs=3) as prsb, \
                 tc.tile_pool(name="prps", bufs=8, space="PSUM") as prps:
                for nq in range(2):
                    pss = [prps.tile([128, CH], f32, tag="ps", name=f"prps{m}")
                           for m in range(KT)]
                    for kt in range(32):
                        panel = wprp.tile([128, E], bf16, tag="w")
                        nc.sync.dma_start(
                            panel[:], w_pr_d[kt * 128:(kt + 1) * 128, :])
                        for m in range(KT):
                            nc.tensor.matmul(
                                pss[m][:], panel[:, m * 128:(m + 1) * 128],
                                gT[:, kt, nq * CH:(nq + 1) * CH],
                                start=(kt == 0), stop=(kt == 31),
                                skip_group_check=True)
                    for m in range(KT):
                        ot = prsb.tile([128, CH], f32, tag="ot")
                        nc.vector.scalar_tensor_tensor(
                            ot[:], pss[m][:], b_pr[:, m, 0:1],
                            x2T[:, m, nq * CH:(nq + 1) * CH],
                            ALU.add, ALU.add)
                        nc.sync.dma_start(
                            out_d[m * 128:(m + 1) * 128, nq * CH:(nq + 1) * CH],
                            ot[:])

    nc.compile()
    return nc


def _host_prep(inputs):
    """Build the 8 per-core input maps."""
    x = np.asarray(inputs["x"], np.float32)
    w_attn = np.asarray(inputs["w_attn"], np.float32).copy()
    w_attn[:, :E] *= 0.125  # fold 1/sqrt(head_dim) into Q
    b_attn = np.asarray(inputs["b_attn"], np.float32).copy()
    b_attn[:E] *= 0.125
    w_attn_bf = np.ascontiguousarray(w_attn.astype(BF))
    b_qk = np.ascontiguousarray(b_attn[:2 * E].reshape(2 * E, 1))
    b_v = np.ascontiguousarray(b_attn[2 * E:].reshape(1, E))
    w_ap_bf = np.ascontiguousarray(np.asarray(inputs["w_attnproj"], np.float32).astype(BF))
    w_fc_bf = np.ascontiguousarray(np.asarray(inputs["w_fc"], np.float32).astype(BF))
    w_pr_bf = np.ascontiguousarray(np.asarray(inputs["w_proj"], np.float32).astype(BF))
    col = lambda v: np.ascontiguousarray(np.asarray(v, np.float32).reshape(-1, 1))
    b_ap = col(inputs["b_attnproj"])
    b_fc = col(inputs["b_fc"])
    b_pr = col(inputs["b_proj"])
    ln1_g = col(inputs["ln1_g"]); ln1_b = col(inputs["ln1_b"])
    ln2_g = col(inputs["ln2_g"]); ln2_b = col(inputs["ln2_b"])

    # static diagonal masks (post-exp multiply): 1 if j >= r*128+p else 0
    j = np.arange(CH)[None, :]
    p = np.arange(128)[:, None]
    dmask = np.stack([np.where(j >= r * 128 + p, 1.0, 0.0) for r in range(4)])
    dmask = np.ascontiguousarray(dmask.astype(BF))

    ON = (1.0, 0.0)
    OFF = (0.0, NEG)
    in_maps = []
    perms = []
    for core in range(8):
        b = core // 2
        z = core % 2
        blocks = [0, 3, 1, 2] if z == 0 else [1, 2, 0, 3]
        perms.append(blocks)
        cols = np.concatenate([np.arange(c * CH, (c + 1) * CH) for c in blocks])
        xT = np.ascontiguousarray(x[b].T[:, cols])
        # slot A: driven block = O1 (perm pos 2); allowed iff block(O1) < block(A)
        sa = ON if blocks[2] < blocks[0] else OFF
        # slot B: driven = A, O1, O2 (perm pos 0, 2, 3) vs chunk B
        sbs = [ON if blocks[i] < blocks[1] else OFF for i in (0, 2, 3)]
        f = np.float32
        in_maps.append({
            "xT": xT,
            "w_attn": w_attn_bf, "b_qk": b_qk, "b_v": b_v,
            "w_ap": w_ap_bf, "b_ap": b_ap,
            "ln1_g": ln1_g, "ln1_b": ln1_b, "ln2_g": ln2_g, "ln2_b": ln2_b,
            "w_fc": w_fc_bf, "b_fc": b_fc, "w_proj": w_pr_bf, "b_proj": b_pr,
            "dmask": dmask,
            "sA_scale": np.full((128, 1), sa[0], f),
            "sA_bias": np.full((128, 1), sa[1], f),
            "sB_scale": np.ascontiguousarray(
                np.tile(np.array([[s for s, _ in sbs]], f), (128, 1))),
            "sB_bias": np.ascontiguousarray(
                np.tile(np.array([[bb for _, bb in sbs]], f), (128, 1))),
        })
    return in_maps, perms


def _run(inputs, trace=False):
    from concourse.bass_utils import run_bass_kernel_spmd

    if "nc" not in _CACHE:
        _CACHE["nc"] = _build_program()
    nc = _CACHE["nc"]
    in_maps, perms = _host_prep(inputs)
    res = run_bass_kernel_spmd(nc, in_maps, list(range(8)), trace=trace)
    x = np.asarray(inputs["x"], np.float32)
    out = np.empty_like(x)
    for core in range(8):
        b = core // 2
        blocks = perms[core]
        oT = res.results[core]["outT"]
        cA, cB = blocks[0], blocks[1]
        out[b, cA * CH:(cA + 1) * CH, :] = oT[:, 0:CH].T
        out[b, cB * CH:(cB + 1) * CH, :] = oT[:, CH:2 * CH].T
    return out, res


def kernel(**inputs) -> np.ndarray:
    out, _ = _run(inputs, trace=False)
    return out



# revision 47
# speedup vs baseline: 1.7511x; 1.3042x over previous
"""GPT block (LN -> causal MHA -> LN -> MLP) on 8 TRN2 NeuronCores.

Sharding: each core owns one (batch, query-chunk-pair). B=4 batches x 2
chunk-pairs = 8 cores. Chunk pairs are zig-zag ({0,3} / {1,2}) over four
512-row chunks of T=2048 so attention work balances. Each core recomputes
K/V for the full sequence locally (no collectives), runs flash-style
attention for its 1024 query rows, then the MLP for the same rows.

All activations live feature-on-partition ("transposed"); the host
pre-transposes x and assembles the output, so no on-chip transposes are
needed. Per-core causality is handled with a block permutation of the
sequence: each core sees [own-chunk-A | own-chunk-B | other | other], so
the program is identical across cores; the diagonal-block masks are
static and full-block allow/deny is driven by per-core exp scale/bias
inputs (exp(0*s - 1e9) = 0 kills forbidden blocks).
"""

import numpy as np
import ml_dtypes

BF = ml_dtypes.bfloat16
F8 = ml_dtypes.float8_e4m3

E = 1024          # embedding
T = 2048          # sequence
B = 4             # batch
NH = 16           # heads
D = 64            # head dim
HID = 4096        # mlp hidden
KT = E // 128     # k-tiles over embedding (8)
CH = 512          # chunk rows
NEG = -1.0e9
EPS = 1e-5

_CACHE = {}


def _build_program():
    import concourse.bass as bass
    import concourse.tile as tile
    from concourse import bacc, mybir

    f32 = mybir.dt.float32
    bf16 = mybir.dt.bfloat16
    f8 = mybir.dt.float8e4
    AF = mybir.ActivationFunctionType
    ALU = mybir.AluOpType
    DR = mybir.MatmulPerfMode.DoubleRow

    nc = bacc.Bacc()

    xT_d = nc.declare_dram_parameter("xT", [E, T], f32, isOutput=False)
    xTb_d = nc.declare_dram_parameter("xTb", [E, T], bf16, isOutput=False)
    w_attn_d = nc.declare_dram_parameter("w_attn", [E, 3 * E], f8, isOutput=False)
    b_qk_d = nc.declare_dram_parameter("b_qk", [2 * E, 1], f32, isOutput=False)
    b_v_d = nc.declare_dram_parameter("b_v", [1, E], f32, isOutput=False)
    w_ap_d = nc.declare_dram_parameter("w_ap", [E, E], bf16, isOutput=False)
    b_ap_d = nc.declare_dram_parameter("b_ap", [E, 1], f32, isOutput=False)
    ln1_g_d = nc.declare_dram_parameter("ln1_g", [E, 1], f32, isOutput=False)
    ln1_b_d = nc.declare_dram_parameter("ln1_b", [E, 1], f32, isOutput=False)
    ln2_g_d = nc.declare_dram_parameter("ln2_g", [E, 1], f32, isOutput=False)
    ln2_b_d = nc.declare_dram_parameter("ln2_b", [E, 1], f32, isOutput=False)
    w_fc_d = nc.declare_dram_parameter("w_fc", [E, HID], bf16, isOutput=False)
    b_fc_d = nc.declare_dram_parameter("b_fc", [HID, 1], f32, isOutput=False)
    w_pr_d = nc.declare_dram_parameter("w_proj", [HID, E], f8, isOutput=False)
    b_pr_d = nc.declare_dram_parameter("b_proj", [1, E], bf16, isOutput=False)
    dmask_d = nc.declare_dram_parameter("dmask", [4, 128, CH], bf16, isOutput=False)
    sA_s_d = nc.declare_dram_parameter("sA_scale", [128, 1], f32, isOutput=False)
    sA_b_d = nc.declare_dram_parameter("sA_bias", [128, 1], f32, isOutput=False)
    sB_s_d = nc.declare_dram_parameter("sB_scale", [128, 3], f32, isOutput=False)
    sB_b_d = nc.declare_dram_parameter("sB_bias", [128, 3], f32, isOutput=False)
    out_d = nc.declare_dram_parameter("outT", [E, 2 * CH], f32, isOutput=True)

    with tile.TileContext(nc) as tc:
        from contextlib import ExitStack

        stack = ExitStack()
        with stack:
            const = stack.enter_context(tc.tile_pool(name="const", bufs=1))

            ones_col_bf = const.tile([128, 1], bf16)
            nc.vector.memset(ones_col_bf[:], 1.0)
            ones_row_f = const.tile([1, 128], f32)
            nc.vector.memset(ones_row_f[:], 1.0)
            ones_row_bf = const.tile([1, 64], bf16)
            nc.vector.memset(ones_row_bf[:], 1.0)
            ones_ch_bf = const.tile([1, CH], bf16)
            nc.vector.memset(ones_ch_bf[:], 1.0)
            eps_t = const.tile([1, 1], f32)
            nc.vector.memset(eps_t[:], EPS)

            dmask = const.tile([128, 4, CH], bf16)
            nc.sync.dma_start(dmask[:], dmask_d.rearrange("v p n -> p v n"))
            sA_s = const.tile([128, 1], f32)
            nc.sync.dma_start(sA_s[:], sA_s_d[:])
            sA_b = const.tile([128, 1], f32)
            nc.sync.dma_start(sA_b[:], sA_b_d[:])
            sB_s = const.tile([128, 3], f32)
            nc.sync.dma_start(sB_s[:], sB_s_d[:])
            sB_b = const.tile([128, 3], f32)
            nc.sync.dma_start(sB_b[:], sB_b_d[:])

            ln1_g = const.tile([128, KT, 1], f32)
            nc.sync.dma_start(ln1_g[:], ln1_g_d.rearrange("(k p) o -> p k o", p=128))
            ln1_b = const.tile([128, KT, 1], f32)
            nc.sync.dma_start(ln1_b[:], ln1_b_d.rearrange("(k p) o -> p k o", p=128))
            ln2_g = const.tile([128, KT, 1], f32)
            nc.sync.dma_start(ln2_g[:], ln2_g_d.rearrange("(k p) o -> p k o", p=128))
            ln2_b = const.tile([128, KT, 1], f32)
            nc.sync.dma_start(ln2_b[:], ln2_b_d.rearrange("(k p) o -> p k o", p=128))
            b_qk = const.tile([128, 16, 1], f32)
            nc.sync.dma_start(b_qk[:], b_qk_d.rearrange("(k p) o -> p k o", p=128))
            b_v = const.tile([1, E], f32)
            nc.sync.dma_start(b_v[:], b_v_d[:])
            b_ap = const.tile([128, KT, 1], f32)
            nc.sync.dma_start(b_ap[:], b_ap_d.rearrange("(k p) o -> p k o", p=128))
            b_fc = const.tile([128, 32, 1], f32)
            nc.sync.dma_start(b_fc[:], b_fc_d.rearrange("(k p) o -> p k o", p=128))
            b_pr = const.tile([1, E], bf16)
            nc.sync.dma_start(b_pr[:], b_pr_d[:])

            def layernorm(dst, src_fn, ncols, g_t, b_t):
                """dst[:, kt, c] = LN over feature dim of src columns.

                src_fn(kt, ch) -> ("dma_bf", dram bf16 AP) | ("sbuf_f32", AP)
                All elementwise work runs in bf16 (DVE 2x mode).
                """
                with tc.tile_pool(name="lnp", bufs=2, space="PSUM") as lnps, \
                     tc.tile_pool(name="lns", bufs=3) as lnsb:
                    for ch in range(ncols // CH):
                        xbfs = []
                        mu_ps = lnps.tile([1, CH], f32, tag="stat")
                        ss_ps = lnps.tile([1, CH], f32, tag="stat")
                        for kt in range(KT):
                            kind, src = src_fn(kt, ch)
                            xbf = lnsb.tile([128, CH], bf16, tag="xbf",
                                            bufs=10)
                            if kind == "dma_bf":
                                nc.sync.dma_start(xbf[:], src)
                            else:
                                nc.vector.tensor_copy(xbf[:], src)
                            xbfs.append(xbf)
                            sq = lnsb.tile([128, CH], bf16, tag="sq")
                            nc.vector.tensor_mul(sq[:], xbf[:], xbf[:])
                            nc.tensor.matmul(mu_ps[:], ones_col_bf[:], xbf[:],
                                             start=(kt == 0), stop=(kt == KT - 1))
                            nc.tensor.matmul(ss_ps[:], ones_col_bf[:], sq[:],
                                             start=(kt == 0), stop=(kt == KT - 1))
                        # rows: musq = (S1/E)^2; var = S2/E - musq;
                        # a = 1/sqrt(var+eps); c = (-S1/E)*a
                        musq = lnsb.tile([1, CH], f32, tag="row", bufs=8)
                        nc.scalar.activation(musq[:], mu_ps[:], AF.Square,
                                             scale=1.0 / E)
                        var = lnsb.tile([1, CH], f32, tag="row", bufs=8)
                        nc.vector.scalar_tensor_tensor(
                            var[:], ss_ps[:], 1.0 / E, musq[:],
                            ALU.mult, ALU.subtract)
                        sd = lnsb.tile([1, CH], f32, tag="row", bufs=8)
                        nc.scalar.activation(sd[:], var[:], AF.Sqrt,
                                             bias=eps_t[:])
                        a_row = lnsb.tile([1, CH], f32, tag="row", bufs=8)
                        nc.vector.reciprocal_approx_fast(out=a_row[:], in_=sd[:])
                        c_row = lnsb.tile([1, CH], f32, tag="row", bufs=8)
                        nc.vector.scalar_tensor_tensor(
                            c_row[:], mu_ps[:], -1.0 / E, a_row[:],
                            ALU.mult, ALU.mult)
                        a_bc = lnps.tile([128, CH], f32, tag="bc")
                        nc.tensor.matmul(a_bc[:], ones_row_f[:], a_row[:],
                                         start=True, stop=True)
                        c_bc = lnps.tile([128, CH], f32, tag="bc")
                        nc.tensor.matmul(c_bc[:], ones_row_f[:], c_row[:],
                                         start=True, stop=True)
                        a_sb = lnsb.tile([128, CH], bf16, tag="asb")
                        nc.vector.tensor_copy(a_sb[:], a_bc[:])
                        c_sb = lnsb.tile([128, CH], bf16, tag="csb")
                        nc.vector.tensor_copy(c_sb[:], c_bc[:])
                        for kt in range(KT):
                            t1 = lnsb.tile([128, CH], bf16, tag="t1")
                            nc.vector.tensor_mul(t1[:], xbfs[kt][:], a_sb[:])
                            t2 = lnsb.tile([128, CH], bf16, tag="t2")
                            nc.vector.tensor_add(t2[:], t1[:], c_sb[:])
                            nc.vector.tensor_scalar(
                                dst[:, kt, ch * CH:(ch + 1) * CH], t2[:],
                                g_t[:, kt, 0:1], b_t[:, kt, 0:1],
                                ALU.mult, ALU.add)

            # ---------------- Phase 1+2: LN1 and QKV ----------------
            ln1_ctx = ExitStack()
            ln1 = ln1_ctx.enter_context(tc.tile_pool(name="ln1", bufs=1))
            ln1_t = ln1.tile([128, KT, T], f8)
            layernorm(ln1_t,
                      lambda kt, ch: ("dma_bf",
                                      xTb_d[kt * 128:(kt + 1) * 128,
                                            ch * CH:(ch + 1) * CH]),
                      T, ln1_g, ln1_b)

            qkv_ctx = ExitStack()
            qkvp = qkv_ctx.enter_context(tc.tile_pool(name="qkvp", bufs=1, side="right"))
            qT = qkvp.tile([128, KT, 2 * CH], bf16)
            kT = qkvp.tile([128, KT, T], bf16)
            v_aug = qkvp.tile([128, 16, NH * 65], bf16)
            v4 = v_aug.rearrange("p m (h w) -> p m h w", h=NH)

            with tc.tile_pool(name="wqk", bufs=2) as wqk_pool, \
                 tc.tile_pool(name="qkps", bufs=4, space="PSUM") as qkps:
                # Q (groups 0..3 cover cols 0..1023), K (4..11 -> 1024..3071)
                # fp8 DoubleRow: contraction pairs of k-tiles, PSUM = 1024x
                # true scale (acts x16, weights x64); descale on evacuation.
                for g in range(8):
                    panel = wqk_pool.tile([128, KT, 256], f8, tag="w")
                    nc.sync.dma_start(
                        panel[:],
                        w_attn_d.rearrange("(k p) n -> p k n", p=128)[
                            :, :, g * 256:(g + 1) * 256])
                    for mm in range(2):
                        mt = 2 * g + mm
                        is_q = mt < 8
                        n_chunks = 2 if is_q else 4
                        dst = qT if is_q else kT
                        dt_idx = mt if is_q else mt - 8
                        for nq in range(n_chunks):
                            ps = qkps.tile([128, CH], f32, tag="ps")
                            for kt in range(0, KT, 2):
                                nc.tensor.matmul(
                                    ps[:],
                                    panel[:, kt:kt + 2, mm * 128:(mm + 1) * 128],
                                    ln1_t[:, kt:kt + 2, nq * CH:(nq + 1) * CH],
                                    start=(kt == 0), stop=(kt == KT - 2),
                                    perf_mode=DR)
                            nc.scalar.activation(
                                dst[:, dt_idx, nq * CH:(nq + 1) * CH], ps[:],
                                AF.Identity, bias=b_qk[:, mt, 0:1],
                                scale=2.0 ** -10)
                # V in natural layout, heads interleaved with a ones column
                for g in range(2):
                    panel = wqk_pool.tile([128, KT, CH], f8, tag="wv")
                    nc.sync.dma_start(
                        panel[:],
                        w_attn_d.rearrange("(k p) n -> p k n", p=128)[
                            :, :, 2 * E + g * CH:2 * E + (g + 1) * CH])
                    bv_bc = qkps.tile([128, CH], f32, tag="bv", bufs=2)
                    nc.tensor.matmul(bv_bc[:], ones_row_f[:],
                                     b_v[:, g * CH:(g + 1) * CH],
                                     start=True, stop=True)
                    bv_sb = wqk_pool.tile([128, CH], f32, tag="bvs")
                    nc.vector.tensor_copy(bv_sb[:], bv_bc[:])
                    bv_sb3 = bv_sb.rearrange("p (h w) -> p h w", h=8)
                    for mv in range(16):
                        ps = qkps.tile([128, CH], f32, tag="ps")
                        for kt in range(0, KT, 2):
                            nc.tensor.matmul(
                                ps[:],
                                ln1_t[:, kt:kt + 2, mv * 128:(mv + 1) * 128],
                                panel[:, kt:kt + 2, :],
                                start=(kt == 0), stop=(kt == KT - 2),
                                perf_mode=DR)
                        ps3 = ps.rearrange("p (h w) -> p h w", h=8)
                        nc.vector.scalar_tensor_tensor(
                            v4[:, mv, g * 8:(g + 1) * 8, 0:64], ps3[:],
                            2.0 ** -10, bv_sb3[:], ALU.mult, ALU.add)
                for mv in range(16):
                    nc.vector.memset(v4[:, mv, :, 64:65], 1.0)
            ln1_ctx.close()

            # ---------------- Phase 3: attention ----------------
            # Head-PAIR processing: heads (2j, 2j+1) live on partition rows
            # 0:64 / 64:128 of feature group j, so their score matmuls use
            # disjoint PE row-groups (tile_position auto-derived) and run
            # concurrently. kv tiles are processed in groups of 2 of the same
            # mask kind so one exp ACTIVATE covers [128, 2*CH] (2 PSUM banks).
            # Diag masks are 0/1 multiplies AFTER exp (cheaper: bf16 2x DVE).
            attn_ctx = ExitStack()
            attnp = attn_ctx.enter_context(tc.tile_pool(name="attnp", bufs=1))
            attnT = attnp.tile([128, KT, 2 * CH], bf16)

            # groups: (kv_t0, kv_t1, kind); kind: ("diag", pair) | ("gate", which, idx)
            groups_a = [(0, 1, ("diag", 0)), (2, 3, ("diag", 1)),
                        (8, 9, ("gate", "A", 0)), (10, 11, ("gate", "A", 0))]
            groups_b = [(4, 5, ("diag", 0)), (6, 7, ("diag", 1)),
                        (0, 1, ("gate", "B", 0)), (2, 3, ("gate", "B", 0)),
                        (8, 9, ("gate", "B", 1)), (10, 11, ("gate", "B", 1)),
                        (12, 13, ("gate", "B", 2)), (14, 15, ("gate", "B", 2))]

            with tc.tile_pool(name="atps", bufs=1, space="PSUM") as atps, \
                 tc.tile_pool(name="atsb", bufs=1) as atsb:
                for slot, groups in ((0, groups_a), (1, groups_b)):
                    qc = slice(slot * CH, (slot + 1) * CH)
                    last = len(groups) - 1

                    def do_scores(j, t0, t1):
                        ss = [atps.tile([128, 2, CH], f32, tag="s",
                                        bufs=3, name=f"s{hh}")
                              for hh in range(2)]
                        for ti, t in enumerate((t0, t1)):
                            for hh in range(2):
                                ro = hh * 64
                                nc.tensor.matmul(
                                    ss[hh][:, ti, :],
                                    kT[ro:ro + 64, j, t * 128:(t + 1) * 128],
                                    qT[ro:ro + 64, j, qc],
                                    start=True, stop=True)
                        return ss

                    def do_exp_av(j, avs, gi, t0, t1, kind, ss):
                        for hh in range(2):
                            es = atsb.tile([128, 2, CH], bf16, tag="es",
                                           bufs=4)
                            if kind[0] == "diag":
                                er = atsb.tile([128, 2, CH], bf16,
                                               tag="er", bufs=2)
                                nc.scalar.activation(er[:], ss[hh][:], AF.Exp)
                                pr = kind[1]
                                nc.vector.tensor_mul(
                                    es[:], er[:],
                                    dmask[:, 2 * pr:2 * pr + 2, :])
                            else:
                                sc = sA_s if kind[1] == "A" else sB_s
                                bi = sA_b if kind[1] == "A" else sB_b
                                idx = kind[2]
                                nc.scalar.activation(
                                    es[:], ss[hh][:], AF.Exp,
                                    bias=bi[:, idx:idx + 1],
                                    scale=sc[:, idx:idx + 1])
                            h = 2 * j + hh
                            for ti, t in enumerate((t0, t1)):
                                nc.tensor.matmul(
                                    avs[hh][:],
                                    v_aug[:, t, h * 65:(h + 1) * 65],
                                    es[:, ti, :],
                                    start=(gi == 0 and ti == 0),
                                    stop=(gi == last and ti == 1))

                    def do_norm(j, avs):
                        # fast reciprocal of the denominator row, PE
                        # broadcast (bc steals an s-tag PSUM slot), scale
                        bct = atps.tile([128, 2, CH], f32, tag="s",
                                        bufs=3, name="bc")
                        for hh in range(2):
                            ro = hh * 64
                            den = atsb.tile([1, CH], f32, tag="den", bufs=2)
                            nc.vector.tensor_copy(den[:], avs[hh][64:65, :])
                            drc = atsb.tile([1, CH], f32, tag="drc", bufs=2)
                            nc.vector.reciprocal_approx_fast(
                                out=drc[:], in_=den[:])
                            drb = atsb.tile([1, CH], bf16, tag="drb", bufs=2)
                            nc.vector.tensor_copy(drb[:], drc[:])
                            nc.tensor.matmul(bct[0:64, hh, :], ones_row_bf[:],
                                             drb[:], start=True, stop=True)
                            bc_sb = atsb.tile([64, CH], bf16, tag="bcs",
                                              bufs=2)
                            nc.vector.tensor_copy(bc_sb[:], bct[0:64, hh, :])
                            nc.vector.tensor_mul(
                                attnT[ro:ro + 64, j, qc],
                                avs[hh][0:64, :], bc_sb[:])

                    # software-pipelined stream: scores run 1-2 groups
                    # ahead of exp/AV; norms deferred one item further
                    work = []
                    norms = []
                    avs_j = {}
                    stream = [(j, gi, grp) for j in range(8)
                              for gi, grp in enumerate(groups)]
                    for j, gi, (t0, t1, kind) in stream:
                        if gi == 0:
                            avs_j[j] = [atps.tile([65, CH], f32, tag="av",
                                                  bufs=2, name=f"av{hh}")
                                        for hh in range(2)]
                        ss = do_scores(j, t0, t1)
                        if norms:
                            do_norm(*norms.pop(0))
                        work.append((j, gi, t0, t1, kind, ss))
                        if len(work) >= 2:
                            jj, gg, tt0, tt1, kk, sss = work.pop(0)
                            do_exp_av(jj, avs_j[jj], gg, tt0, tt1, kk, sss)
                            if gg == last:
                                norms.append((jj, avs_j.pop(jj)))
                    for jj, gg, tt0, tt1, kk, sss in work:
                        do_exp_av(jj, avs_j[jj], gg, tt0, tt1, kk, sss)
                        if gg == last:
                            norms.append((jj, avs_j.pop(jj)))
                    for nrm in norms:
                        do_norm(*nrm)

            qkv_ctx.close()

            # ---------------- Phase 4: attn proj + residual ----------------
            x2p = stack.enter_context(tc.tile_pool(name="x2p", bufs=1, side="right"))
            x2T = x2p.tile([128, KT, 2 * CH], f32)

            with tc.tile_pool(name="app", bufs=1) as app, \
                 tc.tile_pool(name="apsb", bufs=3) as apsb, \
                 tc.tile_pool(name="apps", bufs=3, space="PSUM") as apps:
                w_ap = app.tile([128, KT, E], bf16)
                nc.sync.dma_start(w_ap[:], w_ap_d.rearrange("(k p) n -> p k n", p=128))
                for m in range(KT):
                    for nq in range(2):
                        ps = apps.tile([128, CH], f32, tag="ps")
                        for kt in range(KT):
                            nc.tensor.matmul(
                                ps[:], w_ap[:, kt, m * 128:(m + 1) * 128],
                                attnT[:, kt, nq * CH:(nq + 1) * CH],
                                start=(kt == 0), stop=(kt == KT - 1))
                        xq = apsb.tile([128, CH], f32, tag="xq")
                        nc.sync.dma_start(
                            xq[:], xT_d[m * 128:(m + 1) * 128,
                                        nq * CH:(nq + 1) * CH])
                        nc.vector.scalar_tensor_tensor(
                            x2T[:, m, nq * CH:(nq + 1) * CH], ps[:],
                            b_ap[:, m, 0:1], xq[:], ALU.add, ALU.add)

            attn_ctx.close()

            # ---------------- Phase 5: LN2 ----------------
            h2_ctx = ExitStack()
            h2p = h2_ctx.enter_context(tc.tile_pool(name="h2p", bufs=1))
            h2T = h2p.tile([128, KT, 2 * CH], bf16)
            layernorm(h2T,
                      lambda kt, ch: ("sbuf_f32",
                                      x2T[:, kt, ch * CH:(ch + 1) * CH]),
                      2 * CH, ln2_g, ln2_b)

            # ---------------- Phase 6: FC + GELU ----------------
            gp = stack.enter_context(tc.tile_pool(name="gp", bufs=1, side="right"))
            gT = gp.tile([128, 32, 2 * CH], f8)

            with tc.tile_pool(name="wfcp", bufs=2) as wfcp, \
                 tc.tile_pool(name="fcps", bufs=4, space="PSUM") as fcps:
                for mg in range(8):
                    panel = wfcp.tile([128, KT, CH], bf16, tag="w")
                    nc.sync.dma_start(
                        panel[:],
                        w_fc_d.rearrange("(k p) n -> p k n", p=128)[
                            :, :, mg * CH:(mg + 1) * CH])
                    for mm in range(4):
                        mt = mg * 4 + mm
                        ps2 = fcps.tile([128, 2, CH], f32, tag="ps")
                        for nq in range(2):
                            for kt in range(KT):
                                nc.tensor.matmul(
                                    ps2[:, nq, :],
                                    panel[:, kt, mm * 128:(mm + 1) * 128],
                                    h2T[:, kt, nq * CH:(nq + 1) * CH],
                                    start=(kt == 0), stop=(kt == KT - 1))
                        nc.scalar.activation(
                            gT[:, mt, :], ps2[:],
                            AF.Gelu, bias=b_fc[:, mt, 0:1])

            h2_ctx.close()

            # ---------------- Phase 7: proj + residual + out ----------------
            with tc.tile_pool(name="wprp", bufs=3) as wprp, \
                 tc.tile_pool(name="prsb", bufs=3) as prsb, \
                 tc.tile_pool(name="prps", bufs=8, space="PSUM") as prps:
                for nq in range(2):
                    pss = [prps.tile([128, CH], f32, tag="ps", name=f"prps{m}")
                           for m in range(KT)]
                    for m in range(KT):
                        nc.tensor.matmul(
                            pss[m][:], b_pr[0:1, m * 128:(m + 1) * 128],
                            ones_ch_bf[:], start=True, stop=False,
                            skip_group_check=True)
                    for kt2 in range(16):
                        panel = wprp.tile([128, 2, E], f8, tag="w")
                        nc.sync.dma_start(
                            panel[:],
                            w_pr_d[kt2 * 256:(kt2 + 1) * 256, :].rearrange(
                                "(two p) n -> p two n", p=128))
                        for m in range(KT):
                            nc.tensor.matmul(
                                pss[m][:], panel[:, :, m * 128:(m + 1) * 128],
                                gT[:, 2 * kt2:2 * kt2 + 2,
                                   nq * CH:(nq + 1) * CH],
                                start=False, stop=(kt2 == 15),
                                perf_mode=DR, skip_group_check=True)
                    for m in range(KT):
                        ot = prsb.tile([128, CH], f32, tag="ot")
                        nc.vector.scalar_tensor_tensor(
                            ot[:], pss[m][:], 2.0 ** -6,
                            x2T[:, m, nq * CH:(nq + 1) * CH],
                            ALU.mult, ALU.add)
                        nc.sync.dma_start(
                            out_d[m * 128:(m + 1) * 128, nq * CH:(nq + 1) * CH],
                            ot[:])

    nc.compile()
    return nc


def _host_prep(inputs):
    """Build the 8 per-core input maps.

    fp8 scaling scheme: weights x64, LN outputs x16 (folded into the LN
    gain/bias) -> GEMM PSUM at 1024x (or 64x where the activation input is
    at true scale); descaled during evacuation.
    """
    x = np.asarray(inputs["x"], np.float32)
    w_attn = np.asarray(inputs["w_attn"], np.float32).copy()
    w_attn[:, :E] *= 0.125  # fold 1/sqrt(head_dim) into Q
    b_attn = np.asarray(inputs["b_attn"], np.float32).copy()
    b_attn[:E] *= 0.125
    f8 = lambda w: np.ascontiguousarray(
        (np.asarray(w, np.float32) * 64.0).astype(F8))
    w_attn_f8 = f8(w_attn)
    b_qk = np.ascontiguousarray(b_attn[:2 * E].reshape(2 * E, 1))
    b_v = np.ascontiguousarray(b_attn[2 * E:].reshape(1, E))
    w_ap_bf = np.ascontiguousarray(
        np.asarray(inputs["w_attnproj"], np.float32).astype(BF))
    w_fc_bf = np.ascontiguousarray(
        np.asarray(inputs["w_fc"], np.float32).astype(BF))
    w_pr_f8 = f8(inputs["w_proj"])
    col = lambda v: np.ascontiguousarray(np.asarray(v, np.float32).reshape(-1, 1))
    row64bf = lambda v: np.ascontiguousarray(
        (np.asarray(v, np.float32) * 64.0).reshape(1, -1).astype(BF))
    b_ap = col(inputs["b_attnproj"])
    b_fc = col(inputs["b_fc"])
    b_pr = row64bf(inputs["b_proj"])
    ln1_g = col(inputs["ln1_g"]) * 16.0
    ln1_b = col(inputs["ln1_b"]) * 16.0
    ln2_g = col(inputs["ln2_g"])
    ln2_b = col(inputs["ln2_b"])

    # static diagonal masks (post-exp multiply): 1 if j >= r*128+p else 0
    j = np.arange(CH)[None, :]
    p = np.arange(128)[:, None]
    dmask = np.stack([np.where(j >= r * 128 + p, 1.0, 0.0) for r in range(4)])
    dmask = np.ascontiguousarray(dmask.astype(BF))

    ON = (1.0, 0.0)
    OFF = (0.0, NEG)
    in_maps = []
    perms = []
    for core in range(8):
        b = core // 2
        z = core % 2
        blocks = [0, 3, 1, 2] if z == 0 else [1, 2, 0, 3]
        perms.append(blocks)
        cols = np.concatenate([np.arange(c * CH, (c + 1) * CH) for c in blocks])
        xT = np.ascontiguousarray(x[b].T[:, cols])
        # slot A: driven block = O1 (perm pos 2); allowed iff block(O1) < block(A)
        sa = ON if blocks[2] < blocks[0] else OFF
        # slot B: driven = A, O1, O2 (perm pos 0, 2, 3) vs chunk B
        sbs = [ON if blocks[i] < blocks[1] else OFF for i in (0, 2, 3)]
        f = np.float32
        in_maps.append({
            "xT": xT, "xTb": np.ascontiguousarray(xT.astype(BF)),
            "w_attn": w_attn_f8, "b_qk": b_qk, "b_v": b_v,
            "w_ap": w_ap_bf, "b_ap": b_ap,
            "ln1_g": ln1_g, "ln1_b": ln1_b, "ln2_g": ln2_g, "ln2_b": ln2_b,
            "w_fc": w_fc_bf, "b_fc": b_fc, "w_proj": w_pr_f8, "b_proj": b_pr,
            "dmask": dmask,
            "sA_scale": np.full((128, 1), sa[0], f),
            "sA_bias": np.full((128, 1), sa[1], f),
            "sB_scale": np.ascontiguousarray(
                np.tile(np.array([[s for s, _ in sbs]], f), (128, 1))),
            "sB_bias": np.ascontiguousarray(
                np.tile(np.array([[bb for _, bb in sbs]], f), (128, 1))),
        })
    return in_maps, perms


def _run(inputs, trace=False):
    from concourse.bass_utils import run_bass_kernel_spmd

    if "nc" not in _CACHE:
        _CACHE["nc"] = _build_program()
    nc = _CACHE["nc"]
    in_maps, perms = _host_prep(inputs)
    res = run_bass_kernel_spmd(nc, in_maps, list(range(8)), trace=trace)
    x = np.asarray(inputs["x"], np.float32)
    out = np.empty_like(x)
    for core in range(8):
        b = core // 2
        blocks = perms[core]
        oT = res.results[core]["outT"]
        cA, cB = blocks[0], blocks[1]
        out[b, cA * CH:(cA + 1) * CH, :] = oT[:, 0:CH].T
        out[b, cB * CH:(cB + 1) * CH, :] = oT[:, CH:2 * CH].T
    return out, res


def kernel(**inputs) -> np.ndarray:
    out, _ = _run(inputs, trace=False)
    return out

